# revision 23
# baseline (speedup 1.0000x reference)
"""MixedExpertLayer Trainium2 kernel, v2: host-routed top-2 MoE.

16384 tokens sharded 8 ways (T=2048/core). Routing is resolved on the host:
for each MLP expert e in {0,1} the tokens with nonzero combined weight
c_e = sum_k w_k*[idx_k==e] (~43.75% of tokens) are compacted into a gathered
feature-major input xg_e [H, C]; the device runs the SwiGLU MLP only on those
columns, scales rows by c_e on ACT, and writes compacted outputs Y_e [C, H].
The host scatter-adds Y_e back into the token stream in fp32.

Conv experts (2,3) are cheap and run densely for all tokens — but on the
Vector (e2) and Pool/GpSimd (e3) engines as per-partition-scalar
multiply-accumulate taps over the feature-major halo'd x, keeping the PE
free for MLP matmuls. Their silu runs on ACT, and the weighted combine
c2*y2 + c3*y3 runs on DVE/Pool with host-prebroadcast coefficient tiles.
The conv partial output is stored feature-major [H, T]; the host transposes
and adds.

PE work: only the routed MLP matmuls (bf16, N=512 chunks): ~1536 matmuls/core
vs 3584 in the dense baseline.
"""

import numpy as np
import ml_dtypes

import concourse.bass as bass
import concourse.mybir as mybir
import concourse.tile as tile
from concourse.bass_utils import run_bass_kernel_spmd

B, S, H, I, KTOP, KC = 4, 4096, 1024, 2048, 2, 4
NCORES = 8
T = (B * S) // NCORES          # 2048 tokens per core
TH = T + KC - 1                # 2051 cols with halo
TCH = 512                      # token chunk (matmul N / PSUM bank)
NCHUNK = T // TCH              # 4 conv chunks
HK = H // 128                  # 8 h-chunks
IK = I // 128                  # 16 i-chunks
BF16 = mybir.dt.bfloat16
F32 = mybir.dt.float32
AF = mybir.ActivationFunctionType
MUL = mybir.AluOpType.mult
ADD = mybir.AluOpType.add

# capacity (padded gathered tokens per MLP expert); set by build_in_maps
_ROUTE = {"C": 1024, "lists": None}


def legalize_waits(nc):
    """This walrus build encodes exactly one sync-wait per instruction
    (single NEURON_ISA_TPB_EVENTS slot); Tile emits up to 3 plus a multi-wait
    tail Drain. Split extra waits onto wait-only EventSemaphore carriers
    inserted immediately before the instruction (same engine, same position,
    so no reordering and no deadlock risk)."""
    f = nc.m.functions[0]
    for blk in f.blocks:
        new = []
        for ins in list(blk.instructions):
            si = ins.sync_info
            if si is not None and si.on_wait and len(si.on_wait) > 1:
                best, order = {}, []
                for w in si.on_wait:
                    k = (w.sync_type, w.id, w.wait_mode)
                    if k not in best:
                        best[k] = w
                        order.append(k)
                    elif (w.wait_value or 0) > (best[k].wait_value or 0):
                        best[k] = w
                waits = [best[k] for k in order]
                for j, w in enumerate(waits[:-1]):
                    ev = mybir.InstEventSemaphore(
                        name=f"{ins.name}-lw{j}", engine=ins.engine, ins=[], outs=[],
                    )
                    ev.sync_info = mybir.SyncInfo(on_wait=[w], on_update=[])
                    new.append(ev)
                si.on_wait = [waits[-1]]
                ins.sync_info = si
            new.append(ins)
        blk.instructions = new
    return nc


def build_nc():
    C = _ROUTE["C"]
    NCH_MLP = C // TCH         # chunks per MLP expert
    NTS = TCH // 128           # 4 token subtiles per chunk

    nc = bass.Bass(num_devices=NCORES)
    xf = nc.dram_tensor("xf", [H, TH], BF16, kind="ExternalInput")
    # xg/wg/wu host-repacked so every DMA tile is contiguous per partition
    xgr = nc.dram_tensor("xgr", [2, NCH_MLP, 128, HK, TCH], BF16,
                         kind="ExternalInput")
    wgr = nc.dram_tensor("wgr", [2, IK, 128, HK, 128], BF16,
                         kind="ExternalInput")
    wur = nc.dram_tensor("wur", [2, IK, 128, HK, 128], BF16,
                         kind="ExternalInput")
    wd = nc.dram_tensor("wd", [2, I, H], BF16, kind="ExternalInput")
    cwp = nc.dram_tensor("cwp", [128, 2, HK, KC], F32, kind="ExternalInput")
    cgp = nc.dram_tensor("cgp", [128, 2, C // 128], F32, kind="ExternalInput")
    c23b = nc.dram_tensor("c23b", [128, 2, T], BF16, kind="ExternalInput")
    outc = nc.dram_tensor("outc", [H, T], BF16, kind="ExternalOutput")
    y = nc.dram_tensor("y", [2, C, H], BF16, kind="ExternalOutput")

    xf_t = xf.rearrange("(o p) t -> p o t", p=128)        # [128, HK, TH]
    wd_t = [wd[e].rearrange("(o p) h -> p o h", p=128) for e in range(2)]
    outc_t = outc.rearrange("(o p) t -> p o t", p=128)    # [128, HK, T]

    # (expert, chunk) phase schedule
    phases = [(e, c) for e in range(2) for c in range(NCH_MLP)]

    with tile.TileContext(nc) as tc:
        with (
            tc.tile_pool(name="singles", bufs=1) as singles,
            tc.tile_pool(name="xfp", bufs=2) as xfpool,
            tc.tile_pool(name="wpool", bufs=5) as wpool,
            tc.tile_pool(name="wdpool", bufs=17) as wdpool,
            tc.tile_pool(name="apool", bufs=2) as apool,
            tc.tile_pool(name="accp", bufs=2) as accpool,
            tc.tile_pool(name="ocp", bufs=2) as ocpool,
            tc.tile_pool(name="sgp", bufs=2) as sgpool,
            tc.tile_pool(name="yap", bufs=3) as yapool,
            tc.tile_pool(name="ps", bufs=2, space="PSUM") as ps,
            tc.tile_pool(name="pd", bufs=2, space="PSUM") as pd,
        ):
            # ---- resident state ----
            xg_sb = singles.tile([128, 2, HK, C], BF16)

            def xg_load(pi):
                e, c = phases[pi]
                for hk in range(HK):
                    nc.sync.dma_start(
                        xg_sb[:, e, hk, c * TCH : (c + 1) * TCH],
                        xgr[e, c, :, hk, :])

            xg_load(0)
            cw_sb = singles.tile([128, 2, HK, KC], F32)
            nc.sync.dma_start(cw_sb, cwp[:])
            cg_sb = singles.tile([128, 2, C // 128], F32)
            nc.sync.dma_start(cg_sb, cgp[:])
            c23_sb = singles.tile([128, 2, T], BF16)
            nc.sync.dma_start(c23_sb, c23b[:])

            # ---- conv experts as an op list, interleaved into MLP phases ----
            xft_tiles = {}

            def conv_prefetch(cc):
                if cc >= NCHUNK:
                    return
                t0 = cc * TCH
                xft = xfpool.tile([128, HK, TCH + KC - 1], BF16, tag="xf")
                nc.sync.dma_start(xft[:, 0:4, :], xf_t[:, 0:4, t0 : t0 + TCH + KC - 1])
                nc.sync.dma_start(xft[:, 4:8, :], xf_t[:, 4:8, t0 : t0 + TCH + KC - 1])
                xft_tiles[cc] = xft

            def conv_ops(cc):
                """Closure list for conv chunk cc: taps on DVE, silu on ACT,
                combine on Pool, store via sync DMA."""
                t0 = cc * TCH
                xft = xft_tiles.pop(cc)
                acc = [
                    accpool.tile([128, HK, TCH], BF16, tag="acc2", name="acc2"),
                    accpool.tile([128, HK, TCH], BF16, tag="acc3", name="acc3"),
                ]
                ops = []
                for hk in range(HK):
                    for e in range(2):
                        def tap0(e=e, hk=hk):
                            nc.vector.tensor_scalar(
                                out=acc[e][:, hk, :], in0=xft[:, hk, 0:TCH],
                                scalar1=cw_sb[:, e, hk, 0:1], scalar2=None,
                                op0=MUL)
                        ops.append(tap0)
                        for j in range(1, KC):
                            def tapj(e=e, hk=hk, j=j):
                                nc.vector.scalar_tensor_tensor(
                                    out=acc[e][:, hk, :],
                                    in0=xft[:, hk, j : j + TCH],
                                    scalar=cw_sb[:, e, hk, j : j + 1],
                                    in1=acc[e][:, hk, :], op0=MUL, op1=ADD)
                            ops.append(tapj)
                for e in range(2):
                    for h2 in range(2):
                        def csilu(e=e, h2=h2):
                            nc.scalar.activation(
                                out=acc[e][:, h2 * 4 : (h2 + 1) * 4, :],
                                in_=acc[e][:, h2 * 4 : (h2 + 1) * 4, :],
                                func=AF.Silu)
                        ops.append(csilu)
                oc = ocpool.tile([128, HK, TCH], BF16, tag="oc")
                cb = [c23_sb[:, 0, t0 : t0 + TCH], c23_sb[:, 1, t0 : t0 + TCH]]
                for hk in range(HK):
                    for e in range(2):
                        def cmul(e=e, hk=hk):
                            nc.gpsimd.tensor_mul(
                                acc[e][:, hk, :], acc[e][:, hk, :], cb[e])
                        ops.append(cmul)
                for hk in range(HK):
                    def cadd(hk=hk):
                        nc.gpsimd.tensor_add(
                            oc[:, hk, :], acc[0][:, hk, :], acc[1][:, hk, :])
                    ops.append(cadd)

                def cstore():
                    nc.sync.dma_start(outc_t[:, :, t0 : t0 + TCH], oc)
                ops.append(cstore)
                return ops

            pending = []

            def drain(k):
                for _ in range(min(k, len(pending))):
                    pending.pop(0)()

            conv_prefetch(0)
            for pi, (e, c) in enumerate(phases):
                if pi + 1 < len(phases):
                    xg_load(pi + 1)
                conv_prefetch(pi + 1)
                if pi < NCHUNK:
                    pending.extend(conv_ops(pi))
                # ---- gate/up -> A (feature-major [I, TCH]) ----
                a_sb = apool.tile([128, IK, TCH], BF16, tag="a")
                for i in range(IK):
                    wgt = wpool.tile([128, HK, 128], BF16, tag="wg")
                    nc.sync.dma_start(wgt, wgr[e, i])
                    wut = wpool.tile([128, HK, 128], BF16, tag="wu")
                    nc.sync.dma_start(wut, wur[e, i])
                    psg = ps.tile([128, TCH], F32, tag="pg")
                    psu = ps.tile([128, TCH], F32, tag="pu")
                    for kc in range(HK):
                        nc.tensor.matmul(
                            psg, wgt[:, kc, :],
                            xg_sb[:, e, kc, c * TCH : (c + 1) * TCH],
                            start=(kc == 0), stop=(kc == HK - 1),
                        )
                    for kc in range(HK):
                        nc.tensor.matmul(
                            psu, wut[:, kc, :],
                            xg_sb[:, e, kc, c * TCH : (c + 1) * TCH],
                            start=(kc == 0), stop=(kc == HK - 1),
                        )
                    sg = sgpool.tile([128, TCH], F32, tag="sg")
                    nc.scalar.activation(out=sg, in_=psg, func=AF.Silu)
                    nc.vector.tensor_mul(a_sb[:, i, :], sg, psu)
                    drain(5)

                # ---- down: token-major psum, post-scale by c_e ----
                wds = []
                for kc in range(IK):
                    wdt = wdpool.tile([128, H], BF16, tag="wd")
                    nc.sync.dma_start(wdt, wd_t[e][:, kc, :])
                    wds.append(wdt)
                for ts_ in range(NTS):
                    psd = pd.tile([128, H], F32, tag="pd")
                    for kc in range(IK):
                        lhs = a_sb[:, kc, ts_ * 128 : (ts_ + 1) * 128]
                        nc.tensor.matmul(
                            psd[:, 0:512], lhs, wds[kc][:, 0:512],
                            start=(kc == 0), stop=(kc == IK - 1),
                        )
                        nc.tensor.matmul(
                            psd[:, 512:1024], lhs, wds[kc][:, 512:1024],
                            start=(kc == 0), stop=(kc == IK - 1),
                        )
                    n = c * NTS + ts_
                    ya = yapool.tile([128, H], BF16, tag="ya")
                    nc.scalar.activation(
                        out=ya, in_=psd, func=AF.Copy,
                        scale=cg_sb[:, e, n : n + 1],
                    )
                    row0 = c * TCH + ts_ * 128
                    nc.sync.dma_start(y[e, row0 : row0 + 128, :], ya)
                    drain(5)
                drain(len(pending))
            # any remaining conv chunks (if C < 1024 => fewer MLP phases)
            for cc in range(len(phases), NCHUNK):
                pending.extend(conv_ops(cc))
                conv_prefetch(cc + 1)
                drain(len(pending))
    return legalize_waits(nc)


def _bf16(a):
    return np.asarray(a).astype(ml_dtypes.bfloat16)


def build_in_maps(x, top_k_indices, norm_weights, mlp_gate, mlp_up, mlp_down, conv_w):
    xflat = np.asarray(x, dtype=np.float32).reshape(B * S, H)
    idxflat = np.asarray(top_k_indices).reshape(B * S, KTOP)
    nwflat = np.asarray(norm_weights, dtype=np.float32).reshape(B * S, KTOP)

    # repack gate/up weights into DMA-contiguous tiles:
    # wgr[e, i, p, hk, m] = wg[e, hk*128+p, i*128+m]
    wgr = np.ascontiguousarray(
        _bf16(mlp_gate).reshape(2, HK, 128, IK, 128).transpose(0, 3, 2, 1, 4))
    wur = np.ascontiguousarray(
        _bf16(mlp_up).reshape(2, HK, 128, IK, 128).transpose(0, 3, 2, 1, 4))
    wdb = _bf16(mlp_down)
    # conv weights per partition: cwp[p, e, hc, j] = conv_w[e, hc*128+p, j]
    cw = np.asarray(conv_w, dtype=np.float32).reshape(2, HK, 128, KC)
    cwp = np.ascontiguousarray(cw.transpose(2, 0, 1, 3))  # [128, 2, HK, KC]

    # per-core routing
    cores = []
    for i in range(NCORES):
        lo = i * T
        idx = idxflat[lo : lo + T]
        nw = nwflat[lo : lo + T]
        ce = np.zeros((T, 4), dtype=np.float32)
        rows = np.arange(T)
        for k in range(KTOP):
            np.add.at(ce, (rows, idx[:, k]), nw[:, k])
        lists = [np.nonzero(ce[:, e] != 0.0)[0] for e in range(2)]
        cores.append((lo, ce, lists))

    maxn = max(len(l) for (_, _, ls) in cores for l in ls)
    C = max(TCH, ((maxn + TCH - 1) // TCH) * TCH)
    _ROUTE["C"] = C
    _ROUTE["lists"] = [ls for (_, _, ls) in cores]

    in_maps = []
    for i in range(NCORES):
        lo, ce, lists = cores[i]
        if i % 2 == 0:
            halo = np.zeros((KC - 1, H), dtype=np.float32)
        else:
            halo = xflat[lo - (KC - 1) : lo]
        xh = np.concatenate([halo, xflat[lo : lo + T]], axis=0)  # [T+3, H]
        xf = np.ascontiguousarray(_bf16(xh).T)                   # [H, T+3]

        xg = np.zeros((2, H, C), dtype=ml_dtypes.bfloat16)
        cgp = np.zeros((128, 2, C // 128), dtype=np.float32)
        for e in range(2):
            lst = lists[e]
            n = len(lst)
            xg[e, :, :n] = _bf16(xflat[lo + lst]).T
            cflat = np.zeros(C, dtype=np.float32)
            cflat[:n] = ce[lst, e]
            cgp[:, e, :] = cflat.reshape(C // 128, 128).T
        # xgr[e, c, p, hk, t] = xg[e, hk*128+p, c*TCH+t]
        xgr = np.ascontiguousarray(
            xg.reshape(2, HK, 128, C // TCH, TCH).transpose(0, 3, 2, 1, 4))

        c23b = np.ascontiguousarray(
            np.broadcast_to(ce[:, 2:4].T[:, None, :], (2, 128, T))
            .transpose(1, 0, 2)
        ).astype(ml_dtypes.bfloat16)  # [128, 2, T]

        in_maps.append(
            {"xf": xf, "xgr": xgr, "wgr": wgr, "wur": wur, "wd": wdb,
             "cwp": cwp, "cgp": cgp, "c23b": c23b}
        )
    return in_maps


def assemble(results):
    lists = _ROUTE["lists"]
    out = np.empty((NCORES, T, H), dtype=np.float32)
    for i, r in enumerate(results):
        oc = np.asarray(r["outc"], dtype=np.float32).T     # [T, H]
        yv = r["y"]
        for e in range(2):
            lst = lists[i][e]
            n = len(lst)
            oc[lst] += np.asarray(yv[e, :n], dtype=np.float32)
        out[i] = oc
    return out.reshape(B, S, H)


def kernel(x, top_k_indices, norm_weights, mlp_gate, mlp_up, mlp_down, conv_w):
    in_maps = build_in_maps(
        x, top_k_indices, norm_weights, mlp_gate, mlp_up, mlp_down, conv_w
    )
    nc = build_nc()
    res = run_bass_kernel_spmd(nc, in_maps, core_ids=list(range(NCORES)))
    return assemble(res.results)


# revision 27
# speedup vs baseline: 1.0041x; 1.0041x over previous
"""MixedExpertLayer Trainium2 kernel, v2: host-routed top-2 MoE.

16384 tokens sharded 8 ways (T=2048/core). Routing is resolved on the host:
for each MLP expert e in {0,1} the tokens with nonzero combined weight
c_e = sum_k w_k*[idx_k==e] (~43.75% of tokens) are compacted into a gathered
feature-major input xg_e [H, C]; the device runs the SwiGLU MLP only on those
columns, scales rows by c_e on ACT, and writes compacted outputs Y_e [C, H].
The host scatter-adds Y_e back into the token stream in fp32.

Conv experts (2,3) are cheap and run densely for all tokens — but on the
Vector (e2) and Pool/GpSimd (e3) engines as per-partition-scalar
multiply-accumulate taps over the feature-major halo'd x, keeping the PE
free for MLP matmuls. Their silu runs on ACT, and the weighted combine
c2*y2 + c3*y3 runs on DVE/Pool with host-prebroadcast coefficient tiles.
The conv partial output is stored feature-major [H, T]; the host transposes
and adds.

PE work: only the routed MLP matmuls (bf16, N=512 chunks): ~1536 matmuls/core
vs 3584 in the dense baseline.
"""

import numpy as np
import ml_dtypes

import concourse.bass as bass
import concourse.mybir as mybir
import concourse.tile as tile
from concourse.bass_utils import run_bass_kernel_spmd

B, S, H, I, KTOP, KC = 4, 4096, 1024, 2048, 2, 4
NCORES = 8
T = (B * S) // NCORES          # 2048 tokens per core
TH = T + KC - 1                # 2051 cols with halo
TCH = 512                      # token chunk (matmul N / PSUM bank)
NCHUNK = T // TCH              # 4 conv chunks
HK = H // 128                  # 8 h-chunks
IK = I // 128                  # 16 i-chunks
BF16 = mybir.dt.bfloat16
F32 = mybir.dt.float32
AF = mybir.ActivationFunctionType
MUL = mybir.AluOpType.mult
ADD = mybir.AluOpType.add

# capacity (padded gathered tokens per MLP expert); set by build_in_maps
_ROUTE = {"C": 1024, "lists": None}


def legalize_waits(nc):
    """This walrus build encodes exactly one sync-wait per instruction
    (single NEURON_ISA_TPB_EVENTS slot); Tile emits up to 3 plus a multi-wait
    tail Drain. Split extra waits onto wait-only EventSemaphore carriers
    inserted immediately before the instruction (same engine, same position,
    so no reordering and no deadlock risk)."""
    f = nc.m.functions[0]
    for blk in f.blocks:
        new = []
        for ins in list(blk.instructions):
            si = ins.sync_info
            if si is not None and si.on_wait and len(si.on_wait) > 1:
                best, order = {}, []
                for w in si.on_wait:
                    k = (w.sync_type, w.id, w.wait_mode)
                    if k not in best:
                        best[k] = w
                        order.append(k)
                    elif (w.wait_value or 0) > (best[k].wait_value or 0):
                        best[k] = w
                waits = [best[k] for k in order]
                for j, w in enumerate(waits[:-1]):
                    ev = mybir.InstEventSemaphore(
                        name=f"{ins.name}-lw{j}", engine=ins.engine, ins=[], outs=[],
                    )
                    ev.sync_info = mybir.SyncInfo(on_wait=[w], on_update=[])
                    new.append(ev)
                si.on_wait = [waits[-1]]
                ins.sync_info = si
            new.append(ins)
        blk.instructions = new
    return nc


def build_nc():
    C = _ROUTE["C"]
    NCH_MLP = C // TCH         # chunks per MLP expert
    NTS = TCH // 128           # 4 token subtiles per chunk

    nc = bass.Bass(num_devices=NCORES)
    xf = nc.dram_tensor("xf", [H, TH], BF16, kind="ExternalInput")
    # xg/wg/wu host-repacked so every DMA tile is contiguous per partition
    xgr = nc.dram_tensor("xgr", [2, NCH_MLP, 128, HK, TCH], BF16,
                         kind="ExternalInput")
    wgr = nc.dram_tensor("wgr", [2, IK, 128, HK, 128], BF16,
                         kind="ExternalInput")
    wur = nc.dram_tensor("wur", [2, IK, 128, HK, 128], BF16,
                         kind="ExternalInput")
    wd = nc.dram_tensor("wd", [2, I, H], BF16, kind="ExternalInput")
    cwp = nc.dram_tensor("cwp", [128, 2, HK, KC], F32, kind="ExternalInput")
    cgp = nc.dram_tensor("cgp", [128, 2, C // 128], F32, kind="ExternalInput")
    c23b = nc.dram_tensor("c23b", [128, 2, T], BF16, kind="ExternalInput")
    outc = nc.dram_tensor("outc", [H, T], BF16, kind="ExternalOutput")
    y = nc.dram_tensor("y", [2, C, H], BF16, kind="ExternalOutput")

    xf_t = xf.rearrange("(o p) t -> p o t", p=128)        # [128, HK, TH]
    wd_t = [wd[e].rearrange("(o p) h -> p o h", p=128) for e in range(2)]
    outc_t = outc.rearrange("(o p) t -> p o t", p=128)    # [128, HK, T]

    # (expert, chunk) phase schedule
    phases = [(e, c) for e in range(2) for c in range(NCH_MLP)]

    with tile.TileContext(nc) as tc:
        with (
            tc.tile_pool(name="singles", bufs=1) as singles,
            tc.tile_pool(name="wpool", bufs=5) as wpool,
            tc.tile_pool(name="wdpool", bufs=17) as wdpool,
            tc.tile_pool(name="apool", bufs=2) as apool,
            tc.tile_pool(name="accp", bufs=2) as accpool,
            tc.tile_pool(name="ocp", bufs=2) as ocpool,
            tc.tile_pool(name="sgp", bufs=2) as sgpool,
            tc.tile_pool(name="yap", bufs=3) as yapool,
            tc.tile_pool(name="ps", bufs=2, space="PSUM") as ps,
            tc.tile_pool(name="pd", bufs=2, space="PSUM") as pd,
        ):
            # ---- resident state ----
            xg_sb = singles.tile([128, 2, HK, C], BF16)

            def xg_load(pi):
                e, c = phases[pi]
                for hk in range(HK):
                    nc.sync.dma_start(
                        xg_sb[:, e, hk, c * TCH : (c + 1) * TCH],
                        xgr[e, c, :, hk, :])

            xg_load(0)
            cw_sb = singles.tile([128, 2, HK, KC], F32)
            nc.sync.dma_start(cw_sb, cwp[:])
            cg_sb = singles.tile([128, 2, C // 128], F32)
            nc.sync.dma_start(cg_sb, cgp[:])
            c23_sb = singles.tile([128, 2, T], BF16)
            xf_sb = singles.tile([128, HK, TH], BF16)

            # ---- conv experts: per-hk units over full T, fed through a
            # drain queue so the ops interleave into the MLP phases without
            # head-of-line-blocking any in-order engine ----
            def conv_unit(hk):
                """One hk slab: taps (ACT tap0 + DVE fused mul-add),
                ACT silu, Pool combine with c2/c3, store halves."""
                acc = [
                    accpool.tile([128, T], BF16, tag="acc2", name="acc2"),
                    accpool.tile([128, T], BF16, tag="acc3", name="acc3"),
                ]
                ops = []
                for e in range(2):
                    def tap0(e=e):
                        nc.scalar.activation(
                            out=acc[e], in_=xf_sb[:, hk, 0:T],
                            func=AF.Copy, scale=cw_sb[:, e, hk, 0:1])
                    ops.append(tap0)
                for j in range(1, KC):
                    for e in range(2):
                        def tapj(e=e, j=j):
                            nc.vector.scalar_tensor_tensor(
                                out=acc[e], in0=xf_sb[:, hk, j : j + T],
                                scalar=cw_sb[:, e, hk, j : j + 1],
                                in1=acc[e], op0=MUL, op1=ADD)
                        ops.append(tapj)
                for e in range(2):
                    def csilu(e=e):
                        nc.scalar.activation(out=acc[e], in_=acc[e],
                                             func=AF.Silu)
                    ops.append(csilu)
                for e in range(2):
                    def cmul(e=e):
                        nc.gpsimd.tensor_mul(acc[e], acc[e], c23_sb[:, e, :])
                    ops.append(cmul)
                oc = ocpool.tile([128, T], BF16, tag="oc")
                for h in range(2):
                    def cadd(h=h):
                        nc.gpsimd.tensor_add(
                            oc[:, h * 1024 : (h + 1) * 1024],
                            acc[0][:, h * 1024 : (h + 1) * 1024],
                            acc[1][:, h * 1024 : (h + 1) * 1024])
                    ops.append(cadd)
                for h in range(2):
                    def cstore(h=h):
                        nc.sync.dma_start(
                            outc_t[:, hk, h * 1024 : (h + 1) * 1024],
                            oc[:, h * 1024 : (h + 1) * 1024])
                    ops.append(cstore)
                return ops

            # preamble ops: c23 + xf loads, drained early in phase 0
            pending = [
                lambda: nc.sync.dma_start(c23_sb, c23b[:]),
            ]
            for hk in range(HK):
                pending.append(
                    lambda hk=hk: nc.sync.dma_start(xf_sb[:, hk], xf_t[:, hk]))
            conv_left = list(range(HK))

            nslots = len(phases) * (IK + NTS)
            nops = 9 + HK * 16
            dn = max(2, -(-nops // max(nslots - 10, 1)))

            def drain(k):
                for _ in range(k):
                    if not pending and conv_left:
                        pending.extend(conv_unit(conv_left.pop(0)))
                    if not pending:
                        return
                    pending.pop(0)()

            for pi, (e, c) in enumerate(phases):
                # ---- gate/up -> A (feature-major [I, TCH]) ----
                a_sb = apool.tile([128, IK, TCH], BF16, tag="a")
                wds = []
                for i in range(IK):
                    wgt = wpool.tile([128, HK, 128], BF16, tag="wg")
                    nc.sync.dma_start(wgt, wgr[e, i])
                    wut = wpool.tile([128, HK, 128], BF16, tag="wu")
                    nc.sync.dma_start(wut, wur[e, i])
                    psg = ps.tile([128, TCH], F32, tag="pg")
                    psu = ps.tile([128, TCH], F32, tag="pu")
                    for kc in range(HK):
                        nc.tensor.matmul(
                            psg, wgt[:, kc, :],
                            xg_sb[:, e, kc, c * TCH : (c + 1) * TCH],
                            start=(kc == 0), stop=(kc == HK - 1),
                        )
                    for kc in range(HK):
                        nc.tensor.matmul(
                            psu, wut[:, kc, :],
                            xg_sb[:, e, kc, c * TCH : (c + 1) * TCH],
                            start=(kc == 0), stop=(kc == HK - 1),
                        )
                    sg = sgpool.tile([128, TCH], F32, tag="sg")
                    nc.scalar.activation(out=sg, in_=psg, func=AF.Silu)
                    nc.vector.tensor_mul(a_sb[:, i, :], sg, psu)
                    if i == 8:
                        # mid-phase: prefetch down weights + next xg off the
                        # phase-start critical path
                        for kc in range(IK):
                            wdt = wdpool.tile([128, H], BF16, tag="wd",
                                              name="wdt")
                            nc.sync.dma_start(wdt, wd_t[e][:, kc, :])
                            wds.append(wdt)
                        if pi + 1 < len(phases):
                            xg_load(pi + 1)
                    drain(dn)

                # ---- down: token-major psum, post-scale by c_e ----
                for ts_ in range(NTS):
                    psd = pd.tile([128, H], F32, tag="pd")
                    for kc in range(IK):
                        lhs = a_sb[:, kc, ts_ * 128 : (ts_ + 1) * 128]
                        nc.tensor.matmul(
                            psd[:, 0:512], lhs, wds[kc][:, 0:512],
                            start=(kc == 0), stop=(kc == IK - 1),
                        )
                        nc.tensor.matmul(
                            psd[:, 512:1024], lhs, wds[kc][:, 512:1024],
                            start=(kc == 0), stop=(kc == IK - 1),
                        )
                    n = c * NTS + ts_
                    ya = yapool.tile([128, H], BF16, tag="ya")
                    nc.scalar.activation(
                        out=ya, in_=psd, func=AF.Copy,
                        scale=cg_sb[:, e, n : n + 1],
                    )
                    row0 = c * TCH + ts_ * 128
                    nc.sync.dma_start(y[e, row0 : row0 + 128, :], ya)
                    drain(dn)
            # flush any remaining conv work
            while pending or conv_left:
                drain(16)
    return legalize_waits(nc)


def _bf16(a):
    return np.asarray(a).astype(ml_dtypes.bfloat16)


def build_in_maps(x, top_k_indices, norm_weights, mlp_gate, mlp_up, mlp_down, conv_w):
    xflat = np.asarray(x, dtype=np.float32).reshape(B * S, H)
    idxflat = np.asarray(top_k_indices).reshape(B * S, KTOP)
    nwflat = np.asarray(norm_weights, dtype=np.float32).reshape(B * S, KTOP)

    # repack gate/up weights into DMA-contiguous tiles:
    # wgr[e, i, p, hk, m] = wg[e, hk*128+p, i*128+m]
    wgr = np.ascontiguousarray(
        _bf16(mlp_gate).reshape(2, HK, 128, IK, 128).transpose(0, 3, 2, 1, 4))
    wur = np.ascontiguousarray(
        _bf16(mlp_up).reshape(2, HK, 128, IK, 128).transpose(0, 3, 2, 1, 4))
    wdb = _bf16(mlp_down)
    # conv weights per partition: cwp[p, e, hc, j] = conv_w[e, hc*128+p, j]
    cw = np.asarray(conv_w, dtype=np.float32).reshape(2, HK, 128, KC)
    cwp = np.ascontiguousarray(cw.transpose(2, 0, 1, 3))  # [128, 2, HK, KC]

    # per-core routing
    cores = []
    for i in range(NCORES):
        lo = i * T
        idx = idxflat[lo : lo + T]
        nw = nwflat[lo : lo + T]
        ce = np.zeros((T, 4), dtype=np.float32)
        rows = np.arange(T)
        for k in range(KTOP):
            np.add.at(ce, (rows, idx[:, k]), nw[:, k])
        lists = [np.nonzero(ce[:, e] != 0.0)[0] for e in range(2)]
        cores.append((lo, ce, lists))

    maxn = max(len(l) for (_, _, ls) in cores for l in ls)
    C = max(TCH, ((maxn + TCH - 1) // TCH) * TCH)
    _ROUTE["C"] = C
    _ROUTE["lists"] = [ls for (_, _, ls) in cores]

    in_maps = []
    for i in range(NCORES):
        lo, ce, lists = cores[i]
        if i % 2 == 0:
            halo = np.zeros((KC - 1, H), dtype=np.float32)
        else:
            halo = xflat[lo - (KC - 1) : lo]
        xh = np.concatenate([halo, xflat[lo : lo + T]], axis=0)  # [T+3, H]
        xf = np.ascontiguousarray(_bf16(xh).T)                   # [H, T+3]

        xg = np.zeros((2, H, C), dtype=ml_dtypes.bfloat16)
        cgp = np.zeros((128, 2, C // 128), dtype=np.float32)
        for e in range(2):
            lst = lists[e]
            n = len(lst)
            xg[e, :, :n] = _bf16(xflat[lo + lst]).T
            cflat = np.zeros(C, dtype=np.float32)
            cflat[:n] = ce[lst, e]
            cgp[:, e, :] = cflat.reshape(C // 128, 128).T
        # xgr[e, c, p, hk, t] = xg[e, hk*128+p, c*TCH+t]
        xgr = np.ascontiguousarray(
            xg.reshape(2, HK, 128, C // TCH, TCH).transpose(0, 3, 2, 1, 4))

        c23b = np.ascontiguousarray(
            np.broadcast_to(ce[:, 2:4].T[:, None, :], (2, 128, T))
            .transpose(1, 0, 2)
        ).astype(ml_dtypes.bfloat16)  # [128, 2, T]

        in_maps.append(
            {"xf": xf, "xgr": xgr, "wgr": wgr, "wur": wur, "wd": wdb,
             "cwp": cwp, "cgp": cgp, "c23b": c23b}
        )
    return in_maps


def assemble(results):
    lists = _ROUTE["lists"]
    out = np.empty((NCORES, T, H), dtype=np.float32)
    for i, r in enumerate(results):
        oc = np.asarray(r["outc"], dtype=np.float32).T     # [T, H]
        yv = r["y"]
        for e in range(2):
            lst = lists[i][e]
            n = len(lst)
            oc[lst] += np.asarray(yv[e, :n], dtype=np.float32)
        out[i] = oc
    return out.reshape(B, S, H)


def kernel(x, top_k_indices, norm_weights, mlp_gate, mlp_up, mlp_down, conv_w):
    in_maps = build_in_maps(
        x, top_k_indices, norm_weights, mlp_gate, mlp_up, mlp_down, conv_w
    )
    nc = build_nc()
    res = run_bass_kernel_spmd(nc, in_maps, core_ids=list(range(NCORES)))
    return assemble(res.results)


# revision 29
# speedup vs baseline: 1.1528x; 1.1482x over previous
"""MixedExpertLayer Trainium2 kernel, v8: host-routed, globally balanced top-2 MoE.

Routing is resolved on the host: for each MLP expert e in {0,1} the tokens with
nonzero combined weight c_e = sum_k w_k*[idx_k==e] (~43.75% of all tokens) are
gathered into a compacted feature-major stream. The streams are split EVENLY
across the 8 cores (tokens have no core affinity for the MLP part), so every
core runs exactly ceil(total_e/8) ~ 900 tokens per expert instead of a padded
worst case. The device runs the SwiGLU MLP only on those columns:

  gate/up: psum[i-tile, tok] = sum_h W[h,i]^T x[h, tok]        (feature-major)
  A = silu(g)*u on ACT+DVE
  down (feature-major): psum[h-tile, tok] = sum_i Wd[i,h]^T A[i, tok]
  scaled by c_e via one DVE tensor-tensor with a host-broadcast coefficient row

Y_e is written feature-major [H, C_e]; the host transposes and scatter-adds.

Conv experts (2,3) run densely over each core's home token range on the PE as
diagonal-matrix matmuls (4 taps accumulated in PSUM, diag matrices built
on-device from an identity via ACT per-partition scaling), then ACT silu and a
feature-major weighted combine (in-place 2-operand DVE ops with host-broadcast
c2/c3 rows - these run at 1 elem/cycle unlike 3-operand ops). The conv partial
is stored feature-major [H, T]; host transposes and adds. No PE transposes.

Conv ops are fed through a drain queue that interleaves them between MLP
matmul groups, so no in-order engine stream is blocked by a long foreign op.
"""

import numpy as np
import ml_dtypes

import concourse.bass as bass
import concourse.mybir as mybir
import concourse.tile as tile
from concourse.bass_utils import run_bass_kernel_spmd
from concourse.masks import make_identity

B, S, H, I, KTOP, KC = 4, 4096, 1024, 2048, 2, 4
NCORES = 8
T = (B * S) // NCORES          # 2048 tokens per core
TH = T + KC - 1                # 2051 cols with halo
TCH = 512                      # token chunk (matmul N / PSUM bank)
NCHUNK = T // TCH              # 4 conv chunks
HK = H // 128                  # 8 h-chunks
IK = I // 128                  # 16 i-chunks
BF16 = mybir.dt.bfloat16
F32 = mybir.dt.float32
AF = mybir.ActivationFunctionType
MUL = mybir.AluOpType.mult
ADD = mybir.AluOpType.add

# routing state, set by build_in_maps (device capacities per MLP expert and
# per-(core, expert) global token lists)
_ROUTE = {"C": [1024, 1024], "lists": None}


def legalize_waits(nc):
    """This walrus build encodes exactly one sync-wait per instruction
    (single NEURON_ISA_TPB_EVENTS slot); Tile emits up to 3 plus a multi-wait
    tail Drain. Split extra waits onto wait-only EventSemaphore carriers
    inserted immediately before the instruction (same engine, same position,
    so no reordering and no deadlock risk)."""
    f = nc.m.functions[0]
    for blk in f.blocks:
        new = []
        for ins in list(blk.instructions):
            si = ins.sync_info
            if si is not None and si.on_wait and len(si.on_wait) > 1:
                best, order = {}, []
                for w in si.on_wait:
                    k = (w.sync_type, w.id, w.wait_mode)
                    if k not in best:
                        best[k] = w
                        order.append(k)
                    elif (w.wait_value or 0) > (best[k].wait_value or 0):
                        best[k] = w
                waits = [best[k] for k in order]
                for j, w in enumerate(waits[:-1]):
                    ev = mybir.InstEventSemaphore(
                        name=f"{ins.name}-lw{j}", engine=ins.engine, ins=[], outs=[],
                    )
                    ev.sync_info = mybir.SyncInfo(on_wait=[w], on_update=[])
                    new.append(ev)
                si.on_wait = [waits[-1]]
                ins.sync_info = si
            new.append(ins)
        blk.instructions = new
    return nc


def _windows(Ce):
    """Chunk windows (w0, n) covering Ce tokens in <=TCH pieces."""
    w, out = 0, []
    while w < Ce:
        n = min(TCH, Ce - w)
        out.append((w, n))
        w += n
    return out


def build_nc():
    C0, C1 = _ROUTE["C"]
    nc = bass.Bass(num_devices=NCORES)
    xf = nc.dram_tensor("xf", [H, TH], BF16, kind="ExternalInput")
    xg0 = nc.dram_tensor("xg0", [128, HK, C0], BF16, kind="ExternalInput")
    xg1 = nc.dram_tensor("xg1", [128, HK, C1], BF16, kind="ExternalInput")
    wgr = nc.dram_tensor("wgr", [2, IK, 128, HK, 128], BF16, kind="ExternalInput")
    wur = nc.dram_tensor("wur", [2, IK, 128, HK, 128], BF16, kind="ExternalInput")
    wdr = nc.dram_tensor("wdr", [2, IK, 128, HK, 128], BF16, kind="ExternalInput")
    cwp = nc.dram_tensor("cwp", [128, 2, HK, KC], F32, kind="ExternalInput")
    cgb0 = nc.dram_tensor("cgb0", [128, C0], BF16, kind="ExternalInput")
    cgb1 = nc.dram_tensor("cgb1", [128, C1], BF16, kind="ExternalInput")
    c23b = nc.dram_tensor("c23b", [128, 2, T], BF16, kind="ExternalInput")
    outc = nc.dram_tensor("outc", [H, T], BF16, kind="ExternalOutput")
    yf0 = nc.dram_tensor("yf0", [H, C0], BF16, kind="ExternalOutput")
    yf1 = nc.dram_tensor("yf1", [H, C1], BF16, kind="ExternalOutput")

    xf_t = xf.rearrange("(o p) t -> p o t", p=128)        # [128, HK, TH]
    outc_t = outc.rearrange("(o p) t -> p o t", p=128)    # [128, HK, T]
    yf_t = [y.rearrange("(o p) t -> p o t", p=128) for y in (yf0, yf1)]
    xg_d = [xg0, xg1]
    cgb_d = [cgb0, cgb1]

    phases = [(0, w0, n) for (w0, n) in _windows(C0)] + \
             [(1, w0, n) for (w0, n) in _windows(C1)]

    with tile.TileContext(nc) as tc:
        with (
            tc.tile_pool(name="singles", bufs=1) as singles,
            tc.tile_pool(name="wpool", bufs=5) as wpool,
            tc.tile_pool(name="wdpool", bufs=17) as wdpool,
            tc.tile_pool(name="apool", bufs=2) as apool,
            tc.tile_pool(name="spool", bufs=2) as spool,
            tc.tile_pool(name="ocp", bufs=1) as ocpool,
            tc.tile_pool(name="sgp", bufs=2) as sgpool,
            tc.tile_pool(name="ytp", bufs=3) as ytpool,
            tc.tile_pool(name="ps", bufs=2, space="PSUM") as ps,
            tc.tile_pool(name="pd", bufs=4, space="PSUM") as pd,
        ):
            # ---- tiny resident state first (cw needed for diag build) ----
            cw_sb = singles.tile([128, 2, HK, KC], F32)
            nc.sync.dma_start(cw_sb, cwp[:])
            cgb_sb = [singles.tile([128, Cx], BF16, name=f"cgb{i}")
                      for i, Cx in enumerate((C0, C1))]
            for i in range(2):
                nc.sync.dma_start(cgb_sb[i], cgb_d[i][:])

            xg_sb = [singles.tile([128, HK, Cx], BF16, name=f"xg{i}")
                     for i, Cx in enumerate((C0, C1))]

            def xg_load(e):
                for hk in range(HK):
                    nc.sync.dma_start(xg_sb[e][:, hk], xg_d[e][:, hk])

            xg_load(0)

            # conv diag matrices, built on-device: diag(cw[e2+e, hk*128+p, j])
            ident = singles.tile([128, 128], BF16)
            make_identity(nc, ident)
            diag_sb = singles.tile([128, 2, HK, KC, 128], BF16)
            for e in range(2):
                for hk in range(HK):
                    for j in range(KC):
                        nc.scalar.activation(
                            out=diag_sb[:, e, hk, j, :], in_=ident,
                            func=AF.Copy, scale=cw_sb[:, e, hk, j : j + 1])

            c23_sb = singles.tile([128, 2, T], BF16)
            xf_sb = singles.tile([128, HK, TH], BF16)

            # ---- conv experts: drain-queue units of (chunk, hk) ----
            def conv_unit(cc, hk, oc):
                t0 = cc * TCH
                psc = [None, None]
                s = [None, None]

                def mk_mm(e):
                    def mm(e=e):
                        psc[e] = ps.tile([128, TCH], F32,
                                         tag="pg" if e == 0 else "pu",
                                         name="psc")
                        for j in range(KC):
                            nc.tensor.matmul(
                                psc[e], diag_sb[:, e, hk, j, :],
                                xf_sb[:, hk, t0 + j : t0 + j + TCH],
                                start=(j == 0), stop=(j == KC - 1))
                    return mm

                def mk_silu(e):
                    def op(e=e):
                        s[e] = spool.tile([128, TCH], BF16,
                                          tag=f"s{e}", name="sconv")
                        nc.scalar.activation(out=s[e], in_=psc[e], func=AF.Silu)
                    return op

                def mk_mul(e):
                    def op(e=e):
                        nc.vector.tensor_mul(
                            s[e], s[e], c23_sb[:, e, t0 : t0 + TCH])
                    return op

                def mk_add():
                    def op():
                        nc.vector.tensor_add(oc[:, hk, :], s[0], s[1])
                    return op

                return [mk_mm(0), mk_silu(0), mk_mm(1), mk_silu(1),
                        mk_mul(0), mk_mul(1), mk_add()]

            pending = [lambda: nc.sync.dma_start(c23_sb, c23b[:])]
            for hk in range(HK):
                pending.append(
                    lambda hk=hk: nc.sync.dma_start(xf_sb[:, hk], xf_t[:, hk]))

            conv_left = [(cc, hk) for cc in range(NCHUNK) for hk in range(HK)]
            oc_tiles = {}

            def next_unit():
                cc, hk = conv_left.pop(0)
                if hk == 0:
                    oc_tiles[cc] = ocpool.tile([128, HK, TCH], BF16, tag="oc",
                                               name="oc")
                ops = conv_unit(cc, hk, oc_tiles[cc])
                if hk == HK - 1:
                    oc = oc_tiles.pop(cc)

                    def store(cc=cc, oc=oc):
                        nc.sync.dma_start(
                            outc_t[:, :, cc * TCH : (cc + 1) * TCH], oc)
                    ops.append(store)
                return ops

            nslots = sum(IK + HK for _ in phases)
            nops = 9 + NCHUNK * (HK * 7 + 1)
            dn = max(2, -(-nops // max(nslots - 12, 1)))

            slot_idx = [0]

            def drain(k):
                # during warmup only the preamble DMAs drain, so the first
                # conv matmuls (which wait on xf) don't block the PE stream
                slot_idx[0] += 1
                for _ in range(k):
                    if not pending:
                        if not conv_left or slot_idx[0] <= 12:
                            return
                        pending.extend(next_unit())
                    pending.pop(0)()

            for pi, (e, w0, nw) in enumerate(phases):
                # ---- gate/up -> A (feature-major [I, nw]) ----
                a_sb = apool.tile([128, IK, TCH], BF16, tag="a")
                wds = []
                for i in range(IK):
                    wgt = wpool.tile([128, HK, 128], BF16, tag="wg")
                    nc.sync.dma_start(wgt, wgr[e, i])
                    wut = wpool.tile([128, HK, 128], BF16, tag="wu")
                    nc.sync.dma_start(wut, wur[e, i])
                    psg = ps.tile([128, TCH], F32, tag="pg")
                    psu = ps.tile([128, TCH], F32, tag="pu")
                    for kc in range(HK):
                        nc.tensor.matmul(
                            psg[:, 0:nw], wgt[:, kc, :],
                            xg_sb[e][:, kc, w0 : w0 + nw],
                            start=(kc == 0), stop=(kc == HK - 1))
                    for kc in range(HK):
                        nc.tensor.matmul(
                            psu[:, 0:nw], wut[:, kc, :],
                            xg_sb[e][:, kc, w0 : w0 + nw],
                            start=(kc == 0), stop=(kc == HK - 1))
                    sg = sgpool.tile([128, TCH], F32, tag="sg")
                    nc.scalar.activation(
                        out=sg[:, 0:nw], in_=psg[:, 0:nw], func=AF.Silu)
                    nc.vector.tensor_mul(
                        a_sb[:, i, 0:nw], sg[:, 0:nw], psu[:, 0:nw])
                    if i == 8:
                        # mid-phase: prefetch down weights + next xg stream
                        for kc in range(IK):
                            wdt = wdpool.tile([128, HK, 128], BF16, tag="wd",
                                              name="wdt")
                            nc.sync.dma_start(wdt, wdr[e, kc])
                            wds.append(wdt)
                        if pi == 0:
                            xg_load(1)
                    drain(dn)

                # ---- down, feature-major: psum[h-tile, tok] ----
                for hb in range(HK):
                    psd = pd.tile([128, TCH], F32, tag="pd")
                    for kc in range(IK):
                        nc.tensor.matmul(
                            psd[:, 0:nw], wds[kc][:, hb, :],
                            a_sb[:, kc, 0:nw],
                            start=(kc == 0), stop=(kc == IK - 1))
                    yt = ytpool.tile([128, TCH], BF16, tag="yt")
                    nc.vector.tensor_mul(
                        yt[:, 0:nw], psd[:, 0:nw],
                        cgb_sb[e][:, w0 : w0 + nw])
                    nc.sync.dma_start(
                        yf_t[e][:, hb, w0 : w0 + nw], yt[:, 0:nw])
                    drain(dn)
            # flush any remaining conv work
            while pending or conv_left:
                drain(16)
    return legalize_waits(nc)


def _bf16(a):
    return np.asarray(a).astype(ml_dtypes.bfloat16)


def build_in_maps(x, top_k_indices, norm_weights, mlp_gate, mlp_up, mlp_down, conv_w):
    NT = B * S
    xflat = np.asarray(x, dtype=np.float32).reshape(NT, H)
    idxflat = np.asarray(top_k_indices).reshape(NT, KTOP)
    nwflat = np.asarray(norm_weights, dtype=np.float32).reshape(NT, KTOP)

    # combined per-expert coefficients, global
    ce = np.zeros((NT, 4), dtype=np.float32)
    rows = np.arange(NT)
    for k in range(KTOP):
        np.add.at(ce, (rows, idxflat[:, k]), nwflat[:, k])

    # globally balanced routing: split each expert's token list evenly
    lists = [[], []]
    Cs = [0, 0]
    for e in range(2):
        glst = np.nonzero(ce[:, e] != 0.0)[0]
        lists[e] = np.array_split(glst, NCORES)
        Cs[e] = max(len(l) for l in lists[e])
    _ROUTE["C"] = Cs
    _ROUTE["lists"] = lists

    # weights, repacked so every DMA tile is contiguous per partition
    wgr = np.ascontiguousarray(
        _bf16(mlp_gate).reshape(2, HK, 128, IK, 128).transpose(0, 3, 2, 1, 4))
    wur = np.ascontiguousarray(
        _bf16(mlp_up).reshape(2, HK, 128, IK, 128).transpose(0, 3, 2, 1, 4))
    wdr = np.ascontiguousarray(_bf16(mlp_down).reshape(2, IK, 128, HK, 128))
    cw = np.asarray(conv_w, dtype=np.float32).reshape(2, HK, 128, KC)
    cwp = np.ascontiguousarray(cw.transpose(2, 0, 1, 3))  # [128, 2, HK, KC]

    in_maps = []
    for i in range(NCORES):
        lo = i * T
        if i % 2 == 0:
            halo = np.zeros((KC - 1, H), dtype=np.float32)
        else:
            halo = xflat[lo - (KC - 1) : lo]
        xh = np.concatenate([halo, xflat[lo : lo + T]], axis=0)  # [T+3, H]
        xf = np.ascontiguousarray(_bf16(xh).T)                   # [H, T+3]

        im = {"xf": xf, "wgr": wgr, "wur": wur, "wdr": wdr, "cwp": cwp}
        for e in range(2):
            lst = lists[e][i]
            n = len(lst)
            Cx = Cs[e]
            xg = np.zeros((H, Cx), dtype=ml_dtypes.bfloat16)
            xg[:, :n] = _bf16(xflat[lst]).T
            im[f"xg{e}"] = np.ascontiguousarray(
                xg.reshape(HK, 128, Cx).transpose(1, 0, 2))
            cvec = np.zeros(Cx, dtype=np.float32)
            cvec[:n] = ce[lst, e]
            im[f"cgb{e}"] = np.ascontiguousarray(
                np.broadcast_to(cvec[None, :], (128, Cx))).astype(
                    ml_dtypes.bfloat16)
        im["c23b"] = np.ascontiguousarray(
            np.broadcast_to(ce[lo : lo + T, 2:4].T[:, None, :], (2, 128, T))
            .transpose(1, 0, 2)).astype(ml_dtypes.bfloat16)
        in_maps.append(im)
    return in_maps


def assemble(results):
    lists = _ROUTE["lists"]
    out = np.empty((NT_G := B * S, H), dtype=np.float32)
    for i, r in enumerate(results):
        out[i * T : (i + 1) * T] = np.asarray(r["outc"], dtype=np.float32).T
    for i, r in enumerate(results):
        for e in range(2):
            lst = lists[e][i]
            n = len(lst)
            yv = np.asarray(r[f"yf{e}"], dtype=np.float32)  # [H, C_e]
            out[lst] += yv[:, :n].T
    return out.reshape(B, S, H)


def kernel(x, top_k_indices, norm_weights, mlp_gate, mlp_up, mlp_down, conv_w):
    in_maps = build_in_maps(
        x, top_k_indices, norm_weights, mlp_gate, mlp_up, mlp_down, conv_w
    )
    nc = build_nc()
    res = run_bass_kernel_spmd(nc, in_maps, core_ids=list(range(NCORES)))
    return assemble(res.results)
